# revision 44
# baseline (speedup 1.0000x reference)
"""Trainium2 Bass kernel for nn_BigAttention (weight-norm MLP + softmax-over-k).

Math (per the reference):
    W1e = g1 * W1 / ||W1||_F          [1024, 3072]
    W2e = g2 * W2 / ||W2||_F          [1, 1024]
    hv  = v @ W1e[:, :2048].T         [B,K,N,1024]
    hq  = q @ W1e[:, 2048:].T         [B,K,1024]
    joint  = relu(hv + hq + b1)
    logits = joint @ W2e.T  (+ b2, which cancels in the softmax over k)
    out = softmax(logits, axis=K)     [B,K,N,1]

Sharding: data-parallel over batch, 8 batches per core; weights replicated.

The dominant v-matmuls and the hq matmuls run fp8 e4m3 with
perf_mode=DoubleRow: two 128-deep contraction chunks fuse into one PE
instruction (256-deep contraction per streamed column, ~2x bf16 MACs).
W1 is scaled by S1 host-side so its tiny entries survive e4m3; hq/b1 carry
the same scale and W2 carries 1/S1 (relu commutes with positive scaling).
The one-hot hq-add closers stream hq columns at the PE write floor
(128x1024 broadcast-add needs >=1024 streamed cols/tile) and stay fp32r,
which coexists with the DR mains without weight-path mode thrash.

Per-core device program (rows r = (b_local, k, n) flattened, R = 3456):
  - warmup: a burst of dummy DR matmuls rides the initial DMA wait so the
    PE clock-gate is ramped when real data lands.
  - startup DMAs ride three descriptor-gen rings in consumption order
    (sync = v chunks + qtwq, scalar = W1v groups + w2b, gpsimd = b1b/sel/
    oneh/selT/selTp); chunks 0 and 1 land in small progressive pieces so
    partial tile work starts as bytes arrive, and the last v chunk is
    preloaded into dedicated SBUF right after chunk 1 so the final tiles
    never wait on the ring.  Rings throttle at ~4 in-flight DMAs and share
    the 16 engines unfairly, so every reordering here must be measured.
  - hq[96, 1024] via 8 DoubleRow matmuls; b1 added on the PSUM->SBUF copy.
  - main: per 128-row tile, PSUM[row, hidden 1024] accumulates 8 paired
    v^T-chunk DoubleRow matmuls plus two fp32r one-hot closers adding
    hq[bk(row), :].  Every 512-col matmul streams 1 col/cycle at 2.4GHz
    (~216ns); DR packs 256-deep contraction per column = the fp8 MAC peak.
  - epilogue per tile: one DVE scalar_tensor_tensor computes
    (PSUM max 0) * w2_broadcast with accum_out = per-row sum = the logit.
  - softmax: batches b=0..6 complete UNDER the main loop: head 1 (rt 17,
    bk 0..64) and head 2 (rt 23, bk 64..84 -- tile 23 decides all of b6)
    bounce logits through DRAM, exp + per-(b,n) sums on the PE, then
    reciprocal -> broadcast matmul -> multiply -> rows 0..3024 output DMA,
    all hidden under tiles 24..26.  Only b=7 (rows 3024..3455, tiles
    23..26) remains for the tail, and it never touches DRAM: exp runs
    directly on the [128, tile-col] logit columns, per-n sums and the
    reciprocal gather-back use 1-column PE matmuls with host-built
    n-selection stationaries (the gather also covers tile 23's b6 rows via
    an n-major 1/S_b6 computed at rt 23), a 32-block DVE transpose turns
    the result column-major, and four step-sliced DMAs (3-4 fat
    descriptors each -- a [128,3]-strided write costs ~22ns/descriptor in
    serialized acks) write rows 3008..3455.  Tile 26 splits its closers
    and epilogue per 512-column PSUM half so the last DVE pass is half
    length, and exp(lsA+lsB) fuses the halves via the ACT bias port.

All heavy inputs are host-repacked "partition-major" so every big DMA is 128
contiguous runs (one per partition). DMA issue order matches consumption
order, with the first v/W1v pieces split small so the first real matmul can
start as early as possible.
"""

import ml_dtypes
import numpy as np

import concourse.bacc as bacc
import concourse.mybir as mybir
import concourse.tile as tile
from concourse.bass_utils import run_bass_kernel_spmd

F32 = mybir.dt.float32
NCORES = 8
B, K, N = 64, 12, 36
VD, QD, HID = 2048, 1024, 1024
BL = B // NCORES              # local batches per core
R = BL * K * N                # 3456 rows per core
BK = BL * K                   # 96 (b,k) groups per core
CC = VD // 128                # 16 contraction chunks over v-dim
QC = QD // 128                # 8 contraction chunks over q-dim
RC = 384                      # rows per DMA chunk (9 chunks)
NCH = R // RC
RT = 128                      # rows per PSUM tile
NT = RC // RT
NRT = R // RT                 # 27 row tiles
NRT_A = 18                    # ls_a covers tiles 0..17 = bk 0..64 exactly
B7R0 = 7 * K * N              # 3024: first row of local batch 7
BK84 = 84                     # bk groups fully decided by tiles <= 24

_NC_CACHE = None

MM_DT = mybir.dt.float32r
BF16 = mybir.dt.bfloat16
FP8 = mybir.dt.float8e4
S1 = 2048.0

# cc-chunk DMA granularity per v chunk: chunk 0 lands in five pieces
# (first ones small) so the first matmul pair can start early; later
# chunks use two fat DMAs.
VSPLITS0 = (2, 2, 2, 2, 4, 4)
VSPLITS1 = (2, 2, 4, 4, 4)
VSPLITS = (8, 8)
WSPLITS = (2, 2, 4, 4, 4)     # w1v chunk groups (first ones small)


def _splits_index(splits):
    idx = {}
    base = 0
    for j, g in enumerate(splits):
        for o in range(g):
            idx[base + o] = (j, o)
        base += g
    return idx


def _build_nc():
    nc = bacc.Bacc("TRN2", target_bir_lowering=False, debug=False,
                   num_devices=NCORES)

    DR = mybir.MatmulPerfMode.DoubleRow
    EXPF = mybir.ActivationFunctionType.Exp

    def mm(out, lhsT, rhs, **kw):
        nc.tensor.matmul(out, lhsT, rhs, **kw)

    w1vt = nc.dram_tensor("w1vt", [128, CC, HID], FP8, kind="ExternalInput").ap()
    # qt and W1q^T packed along the free dim: [:, cq, 0:96]=q^T, [:, cq, 96:1120]=W1q^T
    qtwq = nc.dram_tensor("qtwq", [128, QC, BK + HID], FP8, kind="ExternalInput").ap()
    # fp32r one-hot row-selection matrix for the hq-add closers; bf16 was
    # measured to slow the whole PE stream ~20% (FWL/DoubleRow weight-path
    # mode thrash), fp32r keeps the mains at full rate
    oneh_d = nc.dram_tensor("oneh", [BK, R], MM_DT, kind="ExternalInput").ap()
    # small constants: W2e/S1 replicated (bf16), b1*S1 replicated (bf16),
    # softmax selection matrices (fp32)
    w2b = nc.dram_tensor("w2b", [128, HID], BF16, kind="ExternalInput").ap()
    b1b = nc.dram_tensor("b1b", [BK, HID], BF16, kind="ExternalInput").ap()
    sel = nc.dram_tensor("sel", [BK, BL + BK], F32, kind="ExternalInput").ap()
    # b=7 tail n-selection stationaries for tiles 23..26:
    #   selT[p, j, n]  = 1 iff r=(23+j)*128+p >= 3024 and r % 36 == n
    #   selTp[n, j, p] = its transpose (reciprocal gather-back)
    selT_d = nc.dram_tensor("selT", [128, 4, N], F32, kind="ExternalInput").ap()
    selTp_d = nc.dram_tensor("selTp", [100, 4, 128], BF16, kind="ExternalInput").ap()
    # v is split: the first two chunks ride with the weights at the front of
    # the upload order; the bulk uploads last, hidden under early compute.
    vth = nc.dram_tensor("vth", [2, 128, CC, RC], FP8, kind="ExternalInput").ap()
    vtr = nc.dram_tensor("vtr", [NCH - 2, 128, CC, RC], FP8, kind="ExternalInput").ap()
    out = nc.dram_tensor("out", [R], F32, kind="ExternalOutput").ap()

    MAX = mybir.AluOpType.max
    MULT = mybir.AluOpType.mult
    BYPASS = mybir.AluOpType.bypass
    ADD = mybir.AluOpType.add

    vidx0 = _splits_index(VSPLITS0)
    vidx1 = _splits_index(VSPLITS1)
    vidx = _splits_index(VSPLITS)
    widx = _splits_index(WSPLITS)
    vidx_by_npieces = {len(VSPLITS0): vidx0, len(VSPLITS1): vidx1,
                       len(VSPLITS): vidx}

    with tile.TileContext(nc) as tc:
        with tc.tile_pool(name="const", bufs=1) as cpool, \
             tc.tile_pool(name="wv", bufs=1) as wvpool, \
             tc.tile_pool(name="vtp", bufs=3) as vtpool, \
             tc.tile_pool(name="work", bufs=3) as work, \
             tc.tile_pool(name="small", bufs=1) as small, \
             tc.tile_pool(name="dram", bufs=1, space="DRAM") as dpool, \
             tc.tile_pool(name="psum", bufs=4, space="PSUM") as pspool:

            # ---- PE warmup burst: dummy DR matmuls on gpsimd-zeroed SBUF
            # start right after the preamble and ramp the PE p-state while
            # the first real operands upload.  gpsimd memsets come first on
            # the Pool sequencer, before its DMA descriptor gens.
            wdum = small.tile([128, 2, 128], FP8)
            nc.vector.memset(wdum, 0.0)
            xdum = small.tile([128, 2, 512], FP8)
            nc.vector.memset(xdum, 0.0)
            psd = pspool.tile([128, 512], F32, tag="sm", bufs=2)
            for _ in range(14):
                mm(psd, wdum, xdum, start=True, stop=True, perf_mode=DR)

            # ---- startup DMAs across four descriptor-gen queues; per-ring
            # order matches consumption order.
            def vt_chunk_tiles(ch, tagsfx=""):
                src_ap = vth[ch] if ch < 2 else vtr[ch - 2]
                splits = (VSPLITS0 if ch == 0 else
                          VSPLITS1 if ch == 1 else VSPLITS)
                tiles = []
                base = 0
                for j, g in enumerate(splits):
                    t = vtpool.tile([128, g, RC], FP8, tag=f"vt{j}_{g}{tagsfx}")
                    nc.sync.dma_start(out=t, in_=src_ap[:, base:base + g, :])
                    tiles.append(t)
                    base += g
                return tiles

            wv_g = []

            def wv_group(j):
                base = sum(WSPLITS[:j])
                g = WSPLITS[j]
                t = wvpool.tile([128, g, HID], FP8, tag=f"wvg{j}")
                nc.scalar.dma_start(out=t, in_=w1vt[:, base:base + g, :])
                wv_g.append(t)

            vt_cur = vt_chunk_tiles(0)
            for j in range(len(WSPLITS)):
                wv_group(j)

            qtwq_s = cpool.tile([128, QC, BK + HID], FP8)
            nc.sync.dma_start(out=qtwq_s, in_=qtwq)

            vt_next = vt_chunk_tiles(1)

            w2b_s = cpool.tile([128, HID], BF16)
            nc.scalar.dma_start(out=w2b_s, in_=w2b)
            b1b_s = cpool.tile([BK, HID], BF16)
            nc.gpsimd.dma_start(out=b1b_s, in_=b1b)
            sel_s = cpool.tile([BK, BL + BK], F32)
            nc.gpsimd.dma_start(out=sel_s, in_=sel)
            oneh_s = cpool.tile([BK, R], MM_DT)
            nc.gpsimd.dma_start(out=oneh_s, in_=oneh_d)
            selT_s = cpool.tile([128, 4, N], F32)
            nc.gpsimd.dma_start(out=selT_s, in_=selT_d)
            selTp_s = cpool.tile([100, 4, 128], BF16)
            nc.gpsimd.dma_start(out=selTp_s, in_=selTp_d)

            selb_s = sel_s[:, 0:BL]
            selbt_s = sel_s[0:BL, BL:BL + BK]

            # per-row logits, laid out [p, rt] with row = rt*128 + p, split
            # into two tiles so earlier flushes hide under the main loop.
            # 32 columns (StreamTranspose needs 32x32 blocks).
            ls_a = cpool.tile([128, 32], F32)
            nc.vector.memset(ls_a, 0.0)
            ls_b = cpool.tile([128, 32], F32)
            nc.vector.memset(ls_b, 0.0)
            w_ls32 = cpool.tile([128, 32], F32)
            nc.vector.memset(w_ls32, 0.0)
            w_t32 = cpool.tile([128, 32], F32)
            lg = dpool.tile([R], F32)
            lg2 = lg.rearrange("(t p) -> t p", t=NRT, p=128)

            lsA26 = cpool.tile([128, 1], F32)
            lsB26 = cpool.tile([128, 1], F32)

            def flush_logits(ls, ls_t_name, t0, t1):
                # ls[p, t - t0] holds L[t*128 + p] for t in [t0, t1)
                ls_t = cpool.tile([128, 32], F32, name=ls_t_name)
                nc.vector.transpose(ls_t, ls)
                # four rings generate descriptors in parallel (the ~0.7us
                # DIRECT2D gen per DMA is the dominant serial tail cost)
                engs = (nc.sync, nc.scalar, nc.gpsimd, nc.scalar)
                for i in range(4):
                    engs[i].dma_start(
                        out=lg2[t0:t1, 32 * i:32 * i + 32],
                        in_=ls_t[32 * i:32 * i + (t1 - t0), :])

            hq_s = cpool.tile([BK, HID], MM_DT)
            s96 = small.tile([BK, N], F32)
            e96 = small.tile([BK, N], F32)
            sums_ps = pspool.tile([BL, N], F32, tag="sm", bufs=2)

            def softmax_head(bk0, bk1, start, stop):
                # exp + partial per-(b,n) sums for bk rows [bk0, bk1)
                nc.sync.dma_start(
                    out=s96[bk0:bk1, :],
                    in_=lg.rearrange("(bk n) -> bk n", n=N)[bk0:bk1, :])
                nc.scalar.activation(e96[bk0:bk1, :], s96[bk0:bk1, :], EXPF)
                mm(sums_ps, selb_s[bk0:bk1, :], e96[bk0:bk1, :],
                   start=start, stop=stop)

            def emit_vmms(t, ps, half=None):
                # fp8 DoubleRow: each matmul contracts TWO 128-deep v chunks
                # (lhsT [128, 2, 128 rows], rhs [128, 2, 512]).  512-col
                # halves are forced: a matmul out may not cross a PSUM bank.
                halves = (0, 1) if half is None else (half,)
                for cc in range(0, CC, 2):
                    vj, vo = vidx_by_npieces[len(vt_cur)][cc]
                    lhsT = vt_cur[vj][:, vo:vo + 2, t * RT:(t + 1) * RT]
                    wj, wo = widx[cc]
                    wvc = wv_g[wj][:, wo:wo + 2, :]
                    for hh in halves:
                        hs = slice(hh * 512, (hh + 1) * 512)
                        mm(ps[:, hs], lhsT, wvc[:, :, hs],
                           start=(cc == 0), stop=False, perf_mode=DR)

            def emit_closer(rt, ps):
                oh = oneh_s[:, rt * RT:(rt + 1) * RT]
                mm(ps[:, 0:512], oh, hq_s[:, 0:512], start=False, stop=True)
                mm(ps[:, 512:1024], oh, hq_s[:, 512:1024],
                   start=False, stop=True)
                relu_w2 = work.tile([128, HID], F32, tag="relu_w2")
                ls, col = (ls_a, rt) if rt < NRT_A else (ls_b, rt - NRT_A)
                nc.vector.scalar_tensor_tensor(
                    out=relu_w2, in0=ps, scalar=0.0, in1=w2b_s,
                    op0=MAX, op1=MULT,
                    accum_out=ls[:, col:col + 1])
                if rt == NRT_A - 1:
                    # flush + softmax head for bk 0:64 under the main loop
                    flush_logits(ls_a, "ls_ta", 0, NRT_A)
                    softmax_head(0, 64, True, False)
                elif rt == NRT - 4:
                    # tiles 18..23 decide all bk < 84 (b = 0..6): flush +
                    # gather + exp start here; the dependent PE sums issue
                    # at rt 24 when their deps are ready, so they do not
                    # occupy PE wait-queue slots for ~4us (depth is 4; long-
                    # blocked instructions risk head-of-line stalls)
                    flush_logits(ls_b, "ls_tb", NRT_A, NRT - 3)
                    nc.sync.dma_start(
                        out=s96[64:BK84, :],
                        in_=lg.rearrange("(bk n) -> bk n",
                                         n=N)[64:BK84, :])
                    nc.scalar.activation(e96[64:BK84, :], s96[64:BK84, :],
                                         EXPF)

            # ---- chunk 0: v-matmuls emitted wv-GROUP-outer so each arriving
            # W1v piece unlocks matmuls across all 3 tiles (tile-outer would
            # stall until the whole 4.5MB early feed lands); then hq (its DMAs
            # arrive under the v work), then the deferred closers.
            vt_c0 = vt_cur
            ps0 = [pspool.tile([128, HID], F32, tag="ps", bufs=3,
                               name=f"ps0_{t}") for t in range(NT)]
            for wj, wg in enumerate(WSPLITS):
                wbase = sum(WSPLITS[:wj])
                for t in range(NT):
                    for cc in range(wbase, wbase + wg, 2):
                        vj, vo = vidx0[cc]
                        lhsT = vt_c0[vj][:, vo:vo + 2, t * RT:(t + 1) * RT]
                        wvc = wv_g[wj][:, cc - wbase:cc - wbase + 2, :]
                        mm(ps0[t][:, 0:512], lhsT, wvc[:, :, 0:512],
                           start=(cc == 0), stop=False, perf_mode=DR)
                        mm(ps0[t][:, 512:1024], lhsT, wvc[:, :, 512:1024],
                           start=(cc == 0), stop=False, perf_mode=DR)

            # hq[96, 1024] via fp8 DoubleRow over q-chunk pairs (out partition
            # 96 = lhsT free 192 / 2); b1 (scaled) added on the PSUM -> SBUF
            # copy, stored fp32r for the closers.
            for hh in range(2):
                hs = slice(hh * 512, (hh + 1) * 512)
                ps_q = pspool.tile([BK, 512], F32, tag="sm", bufs=2,
                                   name=f"hq_ps{hh}")
                for cq in range(0, QC, 2):
                    mm(ps_q,
                       qtwq_s[:, cq:cq + 2, 0:BK],
                       qtwq_s[:, cq:cq + 2, BK + hh * 512:BK + (hh + 1) * 512],
                       start=(cq == 0), stop=(cq == QC - 2), perf_mode=DR)
                nc.vector.scalar_tensor_tensor(
                    out=hq_s[:, hs], in0=ps_q, scalar=0.0,
                    in1=b1b_s[:, hs], op0=BYPASS, op1=ADD)

            rcp = small.tile([BL, N], F32)
            e_ls = small.tile([128, 4], F32)
            w84 = small.tile([BK84, N], F32)
            # rcp_ext[0:36] = 1/S_b7 (n-major); [36:72] = 1/S_b6 (n-major)
            # for tile 23's b6 rows p 64..79
            rcp_ext = small.tile([100, 1], BF16)
            nc.vector.memset(rcp_ext, 0.0)
            s6T_ps = pspool.tile([N, 1], F32, tag="sm", bufs=2)

            for t in range(NT):
                emit_closer(t, ps0[t])
            vt_cur = vt_next

            # ---- chunks 1..8
            for ch in range(1, NCH):
                if ch == 2:
                    # the last chunk gets dedicated SBUF, issued here (after
                    # vtr[0]) so its 0.79MB neither competes with the
                    # critical early feed nor pushes vtr chunks down the
                    # ring; its normal slot would wait on chunk-6's buffers
                    # and arrive just-in-time (measured ~0.8us tile-26 stall)
                    vt_last = vt_chunk_tiles(NCH - 1, tagsfx="L")
                if ch + 1 < NCH - 1:
                    vt_next = vt_chunk_tiles(ch + 1)
                elif ch + 1 == NCH - 1:
                    vt_next = vt_last
                for t in range(NT):
                    rt = ch * NT + t
                    ps = pspool.tile([128, HID], F32, tag="ps", bufs=3)
                    if rt < NRT - 1:
                        emit_vmms(t, ps)
                        if rt == NRT - 3:
                            # b0..6 sums close + reciprocals + broadcast of
                            # 1/sum: deps (exp from rt 23's gather) are
                            # ready by the time the PE reaches these
                            mm(sums_ps, selb_s[64:BK84, :],
                               e96[64:BK84, :], start=False, stop=True)
                            mm(s6T_ps, e96[64:BK84, :],
                               selb_s[64:BK84, 6:7], start=True, stop=True)
                            nc.vector.reciprocal(rcp[0:7, :],
                                                 sums_ps[0:7, :])
                            with nc.allow_low_precision(
                                    reason="bf16 reciprocal of softmax "
                                           "sums: 0.4% is in budget"):
                                nc.vector.reciprocal(rcp_ext[64:100, :],
                                                     s6T_ps)
                            rexp84_ps = pspool.tile([BK84, N], F32,
                                                    tag="sm", bufs=2)
                            mm(rexp84_ps, sel_s[0:7, BL:BL + BK84],
                               rcp[0:7, :], start=True, stop=True)
                        emit_closer(rt, ps)
                        if rt == NRT - 3:
                            # w for bk 0..84 + its output DMA hide under
                            # tiles 25..26
                            nc.vector.scalar_tensor_tensor(
                                out=w84, in0=e96[0:BK84, :], scalar=0.0,
                                in1=rexp84_ps, op0=BYPASS, op1=MULT)
                            nc.gpsimd.dma_start(
                                out=out[0:2988].rearrange(
                                    "(p f) -> p f", p=83, f=N),
                                in_=w84[0:83, :])
                            nc.gpsimd.dma_start(
                                out=out[2988:3008].rearrange(
                                    "(o f) -> o f", o=1),
                                in_=w84[83:84, 0:20])
                        elif rt == NRT - 2:
                            # exp of tiles 23..25 logit columns hides under
                            # tile 26's mains
                            nc.scalar.activation(
                                e_ls[:, 0:3], ls_b[:, 5:8], EXPF)
                    else:
                        # ---- tile 26: per-half closers/epilogue + the
                        # DRAM-free b=7 softmax tail
                        s7_ps = pspool.tile([N, 1], F32, tag="sm", bufs=2)
                        rcpg_ps = pspool.tile([128, 4], F32, tag="sm", bufs=2)
                        oh = oneh_s[:, rt * RT:(rt + 1) * RT]
                        relu_w2 = work.tile([128, HID], F32, tag="relu_w2")
                        emit_vmms(t, ps, half=0)
                        mm(ps[:, 0:512], oh, hq_s[:, 0:512],
                           start=False, stop=True)
                        nc.vector.scalar_tensor_tensor(
                            out=relu_w2[:, 0:512], in0=ps[:, 0:512],
                            scalar=0.0, in1=w2b_s[:, 0:512],
                            op0=MAX, op1=MULT, accum_out=lsA26)
                        emit_vmms(t, ps, half=1)
                        mm(ps[:, 512:1024], oh, hq_s[:, 512:1024],
                           start=False, stop=True)
                        # per-n sums for tiles 23..25 (deps long ready)
                        for j in range(3):
                            mm(s7_ps, selT_s[:, j, :], e_ls[:, j:j + 1],
                               start=(j == 0), stop=False)
                        nc.vector.scalar_tensor_tensor(
                            out=relu_w2[:, 512:1024], in0=ps[:, 512:1024],
                            scalar=0.0, in1=w2b_s[:, 512:1024],
                            op0=MAX, op1=MULT, accum_out=lsB26)
                        # exp(lsA + lsB) fuses the halves via the ACT bias
                        nc.scalar.activation(e_ls[:, 3:4], lsB26, EXPF,
                                             bias=lsA26)
                        mm(s7_ps, selT_s[:, 3, :], e_ls[:, 3:4],
                           start=False, stop=True)
                        with nc.allow_low_precision(
                                reason="bf16 reciprocal of softmax sums: "
                                       "0.4% is within rel-err budget"):
                            nc.vector.reciprocal(rcp_ext[0:N, :], s7_ps)
                        for j in range(4):
                            mm(rcpg_ps[:, j:j + 1], selTp_s[:, j, :], rcp_ext,
                               start=True, stop=True)
                        nc.vector.scalar_tensor_tensor(
                            out=w_ls32[:, 0:4], in0=e_ls, scalar=0.0,
                            in1=rcpg_ps, op0=BYPASS, op1=MULT)
                        # 32-block transpose so every output DMA is a few
                        # FAT descriptors (a [128,3]-strided write costs
                        # ~22ns/descriptor in serialized acks = 8.5us for
                        # 384 descriptors -- measured)
                        nc.vector.transpose(w_t32, w_ls32)
                        # w_t32[32i + c, pl] = w_ls32[32i + pl, c] = w for
                        # output row 2944 + c*128 + 32i + pl; as 32-element
                        # runs at stride 128 these are step-slices of a
                        # [108, 32] view of out.  Rows < 3008 belong to the
                        # b0..6 path, so i=0/1 skip c=0.
                        ov32 = out.rearrange("(m q) -> m q", m=108, q=32)
                        nc.sync.dma_start(out=ov32[96:105:4],
                                          in_=w_t32[1:4, :])
                        nc.scalar.dma_start(out=ov32[97:106:4],
                                            in_=w_t32[33:36, :])
                        nc.gpsimd.dma_start(out=ov32[94:107:4],
                                            in_=w_t32[64:68, :])
                        nc.scalar.dma_start(out=ov32[95:108:4],
                                            in_=w_t32[96:100, :])
                vt_cur = vt_next

    nc.compile()
    return nc


def _get_nc():
    global _NC_CACHE
    if _NC_CACHE is None:
        _NC_CACHE = _build_nc()
    return _NC_CACHE


def _prepare_in_maps(inputs):
    v = np.asarray(inputs["v"], dtype=np.float32)
    q = np.asarray(inputs["q"], dtype=np.float32)
    W1 = np.asarray(inputs["W1"], dtype=np.float32)
    g1 = np.float64(np.asarray(inputs["g1"]))
    b1 = np.asarray(inputs["b1"], dtype=np.float32)
    W2 = np.asarray(inputs["W2"], dtype=np.float32)
    g2 = np.float64(np.asarray(inputs["g2"]))
    # b2 is a scalar added to every logit -> cancels in softmax over k.

    W1e = ((g1 / np.linalg.norm(W1.astype(np.float64))) * W1).astype(np.float32)
    W2e = ((g2 / np.linalg.norm(W2.astype(np.float64))) * W2).astype(np.float32)

    BF = ml_dtypes.bfloat16
    F8 = ml_dtypes.float8_e4m3   # TRN FP8_EXP4 (max ±240, has inf)
    # partition-major repacks: [..., 128 p, chunk, inner]
    w1vt = np.ascontiguousarray(                       # [128, 16, 1024]
        (W1e[:, :VD] * S1).T.reshape(CC, 128, HID).transpose(1, 0, 2)).astype(F8)
    w1qt = (W1e[:, VD:] * S1).T.reshape(QC, 128, HID).transpose(1, 0, 2)  # [128, 8, 1024]
    r = np.arange(R)
    oneh = (np.arange(BK)[:, None] == (r // N)[None, :]).astype(np.float32)
    selb = (np.arange(BL)[None, :] == (np.arange(BK) // K)[:, None]).astype(np.float32)

    w2bf = np.broadcast_to((W2e.reshape(1, HID) * (1.0 / S1)), (128, HID))
    b1bf = np.broadcast_to((b1.reshape(1, HID) * S1), (BK, HID))
    sel = np.zeros((BK, BL + BK), dtype=np.float32)
    sel[:, 0:BL] = selb
    sel[0:BL, BL:BL + BK] = selb.T

    selT = np.zeros((128, 4, N), dtype=np.float32)
    selTp = np.zeros((100, 4, 128), dtype=np.float32)
    for j in range(4):
        rr = (23 + j) * RT + np.arange(128)
        valid = rr >= B7R0
        pp = np.arange(128)[valid]
        nn = (rr % N)[valid]
        selT[pp, j, nn] = 1.0
        selTp[nn, j, pp] = 1.0
    # tile 23 rows 3008..3023 (p 64..79) belong to b6: gather 1/S_b6 from
    # rcp_ext's second half
    p6 = np.arange(64, 80)
    selTp[64 + ((2944 + p6) % N), 0, p6] = 1.0

    shared = dict(w1vt=w1vt, oneh=oneh, selTp=selTp.astype(BF),
                  w2b=np.ascontiguousarray(w2bf).astype(BF),
                  b1b=np.ascontiguousarray(b1bf).astype(BF), sel=sel,
                  selT=selT)
    in_maps = []
    for c in range(NCORES):
        vl = v[c * BL:(c + 1) * BL].reshape(R, VD)
        # vt[ch, p, cc, r_in_chunk] = v[ch*RC + r, cc*128 + p]
        vt4 = np.ascontiguousarray(
            vl.T.reshape(CC, 128, NCH, RC).transpose(2, 1, 0, 3)).astype(F8)
        ql = q[c * BL:(c + 1) * BL].reshape(BK, QD)
        qt3 = ql.T.reshape(QC, 128, BK).transpose(1, 0, 2)   # [128, 8, 96]
        qtwq = np.concatenate([qt3, w1qt], axis=2)           # [128, 8, 1120]
        in_maps.append(dict(vth=np.ascontiguousarray(vt4[:2]),
                            vtr=np.ascontiguousarray(vt4[2:]),
                            qtwq=np.ascontiguousarray(qtwq).astype(F8),
                            **shared))
    return in_maps


def kernel(**inputs) -> np.ndarray:
    in_maps = _prepare_in_maps(inputs)
    nc = _get_nc()
    res = run_bass_kernel_spmd(nc, in_maps, list(range(NCORES)))
    outs = [res.results[c]["out"].reshape(BL, K, N, 1) for c in range(NCORES)]
    return np.concatenate(outs, axis=0)


# revision 45
# speedup vs baseline: 1.0025x; 1.0025x over previous
"""Trainium2 Bass kernel for nn_BigAttention (weight-norm MLP + softmax-over-k).

Math (per the reference):
    W1e = g1 * W1 / ||W1||_F          [1024, 3072]
    W2e = g2 * W2 / ||W2||_F          [1, 1024]
    hv  = v @ W1e[:, :2048].T         [B,K,N,1024]
    hq  = q @ W1e[:, 2048:].T         [B,K,1024]
    joint  = relu(hv + hq + b1)
    logits = joint @ W2e.T  (+ b2, which cancels in the softmax over k)
    out = softmax(logits, axis=K)     [B,K,N,1]

Sharding: data-parallel over batch, 8 batches per core; weights replicated.

The dominant v-matmuls and the hq matmuls run fp8 e4m3 with
perf_mode=DoubleRow: two 128-deep contraction chunks fuse into one PE
instruction (256-deep contraction per streamed column, ~2x bf16 MACs).
W1 is scaled by S1 host-side so its tiny entries survive e4m3; hq/b1 carry
the same scale and W2 carries 1/S1 (relu commutes with positive scaling).
The one-hot hq-add closers stream hq columns at the PE write floor
(128x1024 broadcast-add needs >=1024 streamed cols/tile) and stay fp32r,
which coexists with the DR mains without weight-path mode thrash.

Per-core device program (rows r = (b_local, k, n) flattened, R = 3456):
  - warmup: a burst of dummy DR matmuls rides the initial DMA wait so the
    PE clock-gate is ramped when real data lands.
  - startup DMAs ride three descriptor-gen rings in consumption order
    (sync = v chunks + qtwq, scalar = W1v groups + w2b, gpsimd = b1b/sel/
    oneh/selT/selTp); chunks 0 and 1 land in small progressive pieces so
    partial tile work starts as bytes arrive, and the last v chunk is
    preloaded into dedicated SBUF right after chunk 1 so the final tiles
    never wait on the ring.  Rings throttle at ~4 in-flight DMAs and share
    the 16 engines unfairly, so every reordering here must be measured.
  - hq[96, 1024] via 8 DoubleRow matmuls; b1 added on the PSUM->SBUF copy.
  - main: per 128-row tile, PSUM[row, hidden 1024] accumulates 8 paired
    v^T-chunk DoubleRow matmuls plus two fp32r one-hot closers adding
    hq[bk(row), :].  Every 512-col matmul streams 1 col/cycle at 2.4GHz
    (~216ns); DR packs 256-deep contraction per column = the fp8 MAC peak.
  - epilogue per tile: one DVE scalar_tensor_tensor computes
    (PSUM max 0) * w2_broadcast with accum_out = per-row sum = the logit.
  - softmax: batches b=0..6 complete UNDER the main loop: head 1 (rt 17,
    bk 0..64) and head 2 (rt 23, bk 64..84 -- tile 23 decides all of b6)
    bounce logits through DRAM, exp + per-(b,n) sums on the PE, then
    reciprocal -> broadcast matmul -> multiply -> rows 0..3024 output DMA,
    all hidden under tiles 24..26.  Only b=7 (rows 3024..3455, tiles
    23..26) remains for the tail, and it never touches DRAM: exp runs
    directly on the [128, tile-col] logit columns, per-n sums and the
    reciprocal gather-back use 1-column PE matmuls with host-built
    n-selection stationaries (the gather also covers tile 23's b6 rows via
    an n-major 1/S_b6 computed at rt 23), a 32-block DVE transpose turns
    the result column-major, and four step-sliced DMAs (3-4 fat
    descriptors each -- a [128,3]-strided write costs ~22ns/descriptor in
    serialized acks) write rows 3008..3455.  Tile 26 splits its closers
    and epilogue per 512-column PSUM half so the last DVE pass is half
    length, and exp(lsA+lsB) fuses the halves via the ACT bias port.

All heavy inputs are host-repacked "partition-major" so every big DMA is 128
contiguous runs (one per partition). DMA issue order matches consumption
order, with the first v/W1v pieces split small so the first real matmul can
start as early as possible.
"""

import ml_dtypes
import numpy as np

import concourse.bacc as bacc
import concourse.mybir as mybir
import concourse.tile as tile
from concourse.bass_utils import run_bass_kernel_spmd

F32 = mybir.dt.float32
NCORES = 8
B, K, N = 64, 12, 36
VD, QD, HID = 2048, 1024, 1024
BL = B // NCORES              # local batches per core
R = BL * K * N                # 3456 rows per core
BK = BL * K                   # 96 (b,k) groups per core
CC = VD // 128                # 16 contraction chunks over v-dim
QC = QD // 128                # 8 contraction chunks over q-dim
RC = 384                      # rows per DMA chunk (9 chunks)
NCH = R // RC
RT = 128                      # rows per PSUM tile
NT = RC // RT
NRT = R // RT                 # 27 row tiles
NRT_A = 18                    # ls_a covers tiles 0..17 = bk 0..64 exactly
B7R0 = 7 * K * N              # 3024: first row of local batch 7
BK84 = 84                     # bk groups fully decided by tiles <= 24

_NC_CACHE = None

MM_DT = mybir.dt.float32r
BF16 = mybir.dt.bfloat16
FP8 = mybir.dt.float8e4
S1 = 2048.0

# cc-chunk DMA granularity per v chunk: chunk 0 lands in five pieces
# (first ones small) so the first matmul pair can start early; later
# chunks use two fat DMAs.
VSPLITS0 = (2, 2, 2, 2, 4, 4)
VSPLITS1 = (2, 2, 4, 4, 4)
VSPLITS = (8, 8)
WSPLITS = (2, 2, 4, 4, 4)     # w1v chunk groups (first ones small)


def _splits_index(splits):
    idx = {}
    base = 0
    for j, g in enumerate(splits):
        for o in range(g):
            idx[base + o] = (j, o)
        base += g
    return idx


def _build_nc():
    nc = bacc.Bacc("TRN2", target_bir_lowering=False, debug=False,
                   num_devices=NCORES)

    DR = mybir.MatmulPerfMode.DoubleRow
    EXPF = mybir.ActivationFunctionType.Exp

    def mm(out, lhsT, rhs, **kw):
        nc.tensor.matmul(out, lhsT, rhs, **kw)

    w1vt = nc.dram_tensor("w1vt", [128, CC, HID], FP8, kind="ExternalInput").ap()
    # qt and W1q^T packed along the free dim: [:, cq, 0:96]=q^T, [:, cq, 96:1120]=W1q^T
    qtwq = nc.dram_tensor("qtwq", [128, QC, BK + HID], FP8, kind="ExternalInput").ap()
    # fp32r one-hot row-selection matrix for the hq-add closers; bf16 was
    # measured to slow the whole PE stream ~20% (FWL/DoubleRow weight-path
    # mode thrash), fp32r keeps the mains at full rate
    oneh_d = nc.dram_tensor("oneh", [BK, R], MM_DT, kind="ExternalInput").ap()
    # small constants: W2e/S1 replicated (bf16), b1*S1 replicated (bf16),
    # softmax selection matrices (fp32)
    w2b = nc.dram_tensor("w2b", [128, HID], BF16, kind="ExternalInput").ap()
    b1b = nc.dram_tensor("b1b", [BK, HID], BF16, kind="ExternalInput").ap()
    sel = nc.dram_tensor("sel", [BK, BL + BK], F32, kind="ExternalInput").ap()
    # b=7 tail n-selection stationaries for tiles 23..26:
    #   selT[p, j, n]  = 1 iff r=(23+j)*128+p >= 3024 and r % 36 == n
    #   selTp[n, j, p] = its transpose (reciprocal gather-back)
    selT_d = nc.dram_tensor("selT", [128, 4, N], F32, kind="ExternalInput").ap()
    selTp_d = nc.dram_tensor("selTp", [100, 4, 128], BF16, kind="ExternalInput").ap()
    # v is split: the first two chunks ride with the weights at the front of
    # the upload order; the bulk uploads last, hidden under early compute.
    vth = nc.dram_tensor("vth", [2, 128, CC, RC], FP8, kind="ExternalInput").ap()
    vtr = nc.dram_tensor("vtr", [NCH - 2, 128, CC, RC], FP8, kind="ExternalInput").ap()
    out = nc.dram_tensor("out", [R], F32, kind="ExternalOutput").ap()

    MAX = mybir.AluOpType.max
    MULT = mybir.AluOpType.mult
    BYPASS = mybir.AluOpType.bypass
    ADD = mybir.AluOpType.add

    vidx0 = _splits_index(VSPLITS0)
    vidx1 = _splits_index(VSPLITS1)
    vidx = _splits_index(VSPLITS)
    widx = _splits_index(WSPLITS)
    vidx_by_npieces = {len(VSPLITS0): vidx0, len(VSPLITS1): vidx1,
                       len(VSPLITS): vidx}

    with tile.TileContext(nc) as tc:
        with tc.tile_pool(name="const", bufs=1) as cpool, \
             tc.tile_pool(name="wv", bufs=1) as wvpool, \
             tc.tile_pool(name="vtp", bufs=3) as vtpool, \
             tc.tile_pool(name="work", bufs=3) as work, \
             tc.tile_pool(name="small", bufs=1) as small, \
             tc.tile_pool(name="dram", bufs=1, space="DRAM") as dpool, \
             tc.tile_pool(name="psum", bufs=4, space="PSUM") as pspool:

            # ---- PE warmup burst: dummy DR matmuls on gpsimd-zeroed SBUF
            # start right after the preamble and ramp the PE p-state while
            # the first real operands upload.  gpsimd memsets come first on
            # the Pool sequencer, before its DMA descriptor gens.
            wdum = small.tile([128, 2, 128], FP8)
            nc.vector.memset(wdum, 0.0)
            xdum = small.tile([128, 2, 512], FP8)
            nc.vector.memset(xdum, 0.0)
            psd = pspool.tile([128, 512], F32, tag="sm", bufs=2)
            for _ in range(14):
                mm(psd, wdum, xdum, start=True, stop=True, perf_mode=DR)

            # ---- startup DMAs across four descriptor-gen queues; per-ring
            # order matches consumption order.
            def vt_chunk_tiles(ch, tagsfx=""):
                src_ap = vth[ch] if ch < 2 else vtr[ch - 2]
                splits = (VSPLITS0 if ch == 0 else
                          VSPLITS1 if ch == 1 else VSPLITS)
                tiles = []
                base = 0
                for j, g in enumerate(splits):
                    t = vtpool.tile([128, g, RC], FP8, tag=f"vt{j}_{g}{tagsfx}")
                    nc.sync.dma_start(out=t, in_=src_ap[:, base:base + g, :])
                    tiles.append(t)
                    base += g
                return tiles

            wv_g = []

            def wv_group(j):
                base = sum(WSPLITS[:j])
                g = WSPLITS[j]
                t = wvpool.tile([128, g, HID], FP8, tag=f"wvg{j}")
                nc.scalar.dma_start(out=t, in_=w1vt[:, base:base + g, :])
                wv_g.append(t)

            vt_cur = vt_chunk_tiles(0)
            for j in range(len(WSPLITS)):
                wv_group(j)

            qtwq_s = cpool.tile([128, QC, BK + HID], FP8)
            nc.sync.dma_start(out=qtwq_s, in_=qtwq)

            vt_next = vt_chunk_tiles(1)

            w2b_s = cpool.tile([128, HID], BF16)
            nc.scalar.dma_start(out=w2b_s, in_=w2b)
            b1b_s = cpool.tile([BK, HID], BF16)
            nc.gpsimd.dma_start(out=b1b_s, in_=b1b)
            sel_s = cpool.tile([BK, BL + BK], F32)
            nc.gpsimd.dma_start(out=sel_s, in_=sel)
            oneh_s = cpool.tile([BK, R], MM_DT)
            nc.gpsimd.dma_start(out=oneh_s, in_=oneh_d)
            selT_s = cpool.tile([128, 4, N], F32)
            nc.gpsimd.dma_start(out=selT_s, in_=selT_d)
            selTp_s = cpool.tile([100, 4, 128], BF16)
            nc.gpsimd.dma_start(out=selTp_s, in_=selTp_d)

            selb_s = sel_s[:, 0:BL]
            selbt_s = sel_s[0:BL, BL:BL + BK]

            # per-row logits, laid out [p, rt] with row = rt*128 + p, split
            # into two tiles so earlier flushes hide under the main loop.
            # 32 columns (StreamTranspose needs 32x32 blocks).
            ls_a = cpool.tile([128, 32], F32)
            nc.vector.memset(ls_a, 0.0)
            ls_b = cpool.tile([128, 32], F32)
            nc.vector.memset(ls_b, 0.0)
            w_ls32 = cpool.tile([128, 32], F32)
            nc.vector.memset(w_ls32, 0.0)
            w_t32 = cpool.tile([128, 32], F32)
            lg = dpool.tile([R], F32)
            lg2 = lg.rearrange("(t p) -> t p", t=NRT, p=128)

            lsA26 = cpool.tile([128, 1], F32)
            lsB26 = cpool.tile([128, 1], F32)

            def flush_logits(ls, ls_t_name, t0, t1):
                # ls[p, t - t0] holds L[t*128 + p] for t in [t0, t1)
                ls_t = cpool.tile([128, 32], F32, name=ls_t_name)
                nc.vector.transpose(ls_t, ls)
                # four rings generate descriptors in parallel (the ~0.7us
                # DIRECT2D gen per DMA is the dominant serial tail cost)
                engs = (nc.sync, nc.scalar, nc.gpsimd, nc.scalar)
                for i in range(4):
                    engs[i].dma_start(
                        out=lg2[t0:t1, 32 * i:32 * i + 32],
                        in_=ls_t[32 * i:32 * i + (t1 - t0), :])

            hq_s = cpool.tile([BK, HID], MM_DT)
            s96 = small.tile([BK, N], F32)
            e96 = small.tile([BK, N], F32)
            sums_ps = pspool.tile([BL, N], F32, tag="sm", bufs=2)

            def softmax_head(bk0, bk1, start, stop):
                # exp + partial per-(b,n) sums for bk rows [bk0, bk1)
                nc.sync.dma_start(
                    out=s96[bk0:bk1, :],
                    in_=lg.rearrange("(bk n) -> bk n", n=N)[bk0:bk1, :])
                nc.scalar.activation(e96[bk0:bk1, :], s96[bk0:bk1, :], EXPF)
                mm(sums_ps, selb_s[bk0:bk1, :], e96[bk0:bk1, :],
                   start=start, stop=stop)

            def emit_vmms(t, ps, half=None):
                # fp8 DoubleRow: each matmul contracts TWO 128-deep v chunks
                # (lhsT [128, 2, 128 rows], rhs [128, 2, 512]).  512-col
                # halves are forced: a matmul out may not cross a PSUM bank.
                halves = (0, 1) if half is None else (half,)
                for cc in range(0, CC, 2):
                    vj, vo = vidx_by_npieces[len(vt_cur)][cc]
                    lhsT = vt_cur[vj][:, vo:vo + 2, t * RT:(t + 1) * RT]
                    wj, wo = widx[cc]
                    wvc = wv_g[wj][:, wo:wo + 2, :]
                    for hh in halves:
                        hs = slice(hh * 512, (hh + 1) * 512)
                        mm(ps[:, hs], lhsT, wvc[:, :, hs],
                           start=(cc == 0), stop=False, perf_mode=DR)

            def emit_closer(rt, ps):
                oh = oneh_s[:, rt * RT:(rt + 1) * RT]
                mm(ps[:, 0:512], oh, hq_s[:, 0:512], start=False, stop=True)
                mm(ps[:, 512:1024], oh, hq_s[:, 512:1024],
                   start=False, stop=True)
                relu_w2 = work.tile([128, HID], F32, tag="relu_w2")
                ls, col = (ls_a, rt) if rt < NRT_A else (ls_b, rt - NRT_A)
                nc.vector.scalar_tensor_tensor(
                    out=relu_w2, in0=ps, scalar=0.0, in1=w2b_s,
                    op0=MAX, op1=MULT,
                    accum_out=ls[:, col:col + 1])
                if rt == NRT_A - 1:
                    # flush + softmax head for bk 0:64 under the main loop
                    flush_logits(ls_a, "ls_ta", 0, NRT_A)
                    softmax_head(0, 64, True, False)
                elif rt == NRT - 4:
                    # tiles 18..23 decide all bk < 84 (b = 0..6); the second
                    # head CLOSES the b0..6 sums so their softmax + output
                    # writes hide under tiles 24..26
                    flush_logits(ls_b, "ls_tb", NRT_A, NRT - 3)
                    softmax_head(64, BK84, False, True)
                    nc.vector.reciprocal(rcp[0:7, :], sums_ps[0:7, :])
                    # n-major 1/S_b6 for the tail's tile-23 b6 rows: one tiny
                    # matmul (e96 stationary, b6-indicator moving) + recip
                    mm(s6T_ps, e96[64:BK84, :], selb_s[64:BK84, 6:7],
                       start=True, stop=True)
                    with nc.allow_low_precision(
                            reason="bf16 reciprocal of softmax sums: "
                                   "0.4% is within rel-err budget"):
                        nc.vector.reciprocal(rcp_ext[64:100, :], s6T_ps)

            # ---- chunk 0: v-matmuls emitted wv-GROUP-outer so each arriving
            # W1v piece unlocks matmuls across all 3 tiles (tile-outer would
            # stall until the whole 4.5MB early feed lands); then hq (its DMAs
            # arrive under the v work), then the deferred closers.
            vt_c0 = vt_cur
            ps0 = [pspool.tile([128, HID], F32, tag="ps", bufs=3,
                               name=f"ps0_{t}") for t in range(NT)]
            for wj, wg in enumerate(WSPLITS):
                wbase = sum(WSPLITS[:wj])
                for t in range(NT):
                    for cc in range(wbase, wbase + wg, 2):
                        vj, vo = vidx0[cc]
                        lhsT = vt_c0[vj][:, vo:vo + 2, t * RT:(t + 1) * RT]
                        wvc = wv_g[wj][:, cc - wbase:cc - wbase + 2, :]
                        mm(ps0[t][:, 0:512], lhsT, wvc[:, :, 0:512],
                           start=(cc == 0), stop=False, perf_mode=DR)
                        mm(ps0[t][:, 512:1024], lhsT, wvc[:, :, 512:1024],
                           start=(cc == 0), stop=False, perf_mode=DR)

            # hq[96, 1024] via fp8 DoubleRow over q-chunk pairs (out partition
            # 96 = lhsT free 192 / 2); b1 (scaled) added on the PSUM -> SBUF
            # copy, stored fp32r for the closers.
            for hh in range(2):
                hs = slice(hh * 512, (hh + 1) * 512)
                ps_q = pspool.tile([BK, 512], F32, tag="sm", bufs=2,
                                   name=f"hq_ps{hh}")
                for cq in range(0, QC, 2):
                    mm(ps_q,
                       qtwq_s[:, cq:cq + 2, 0:BK],
                       qtwq_s[:, cq:cq + 2, BK + hh * 512:BK + (hh + 1) * 512],
                       start=(cq == 0), stop=(cq == QC - 2), perf_mode=DR)
                nc.vector.scalar_tensor_tensor(
                    out=hq_s[:, hs], in0=ps_q, scalar=0.0,
                    in1=b1b_s[:, hs], op0=BYPASS, op1=ADD)

            rcp = small.tile([BL, N], F32)
            e_ls = small.tile([128, 4], F32)
            w84 = small.tile([BK84, N], F32)
            # rcp_ext[0:36] = 1/S_b7 (n-major); [36:72] = 1/S_b6 (n-major)
            # for tile 23's b6 rows p 64..79
            rcp_ext = small.tile([100, 1], BF16)
            nc.vector.memset(rcp_ext, 0.0)
            s6T_ps = pspool.tile([N, 1], F32, tag="sm", bufs=2)

            for t in range(NT):
                emit_closer(t, ps0[t])
            vt_cur = vt_next

            # ---- chunks 1..8
            for ch in range(1, NCH):
                if ch == 2:
                    # the last chunk gets dedicated SBUF, issued here (after
                    # vtr[0]) so its 0.79MB neither competes with the
                    # critical early feed nor pushes vtr chunks down the
                    # ring; its normal slot would wait on chunk-6's buffers
                    # and arrive just-in-time (measured ~0.8us tile-26 stall)
                    vt_last = vt_chunk_tiles(NCH - 1, tagsfx="L")
                if ch + 1 < NCH - 1:
                    vt_next = vt_chunk_tiles(ch + 1)
                elif ch + 1 == NCH - 1:
                    vt_next = vt_last
                for t in range(NT):
                    rt = ch * NT + t
                    ps = pspool.tile([128, HID], F32, tag="ps", bufs=3)
                    if rt < NRT - 1:
                        emit_vmms(t, ps)
                        if rt == NRT - 3:
                            # b0..6 broadcast of 1/sum: issued after tile
                            # 24's mains so the deps (reciprocal at rt 23)
                            # are ready by the time the PE reaches it
                            rexp84_ps = pspool.tile([BK84, N], F32,
                                                    tag="sm", bufs=2)
                            mm(rexp84_ps, sel_s[0:7, BL:BL + BK84],
                               rcp[0:7, :], start=True, stop=True)
                        emit_closer(rt, ps)
                        if rt == NRT - 3:
                            # w for bk 0..84 + its output DMA hide under
                            # tiles 25..26
                            nc.vector.scalar_tensor_tensor(
                                out=w84, in0=e96[0:BK84, :], scalar=0.0,
                                in1=rexp84_ps, op0=BYPASS, op1=MULT)
                            nc.gpsimd.dma_start(
                                out=out[0:2988].rearrange(
                                    "(p f) -> p f", p=83, f=N),
                                in_=w84[0:83, :])
                            nc.gpsimd.dma_start(
                                out=out[2988:3008].rearrange(
                                    "(o f) -> o f", o=1),
                                in_=w84[83:84, 0:20])
                        elif rt == NRT - 2:
                            # exp of tiles 23..25 logit columns hides under
                            # tile 26's mains
                            nc.scalar.activation(
                                e_ls[:, 0:3], ls_b[:, 5:8], EXPF)
                    else:
                        # ---- tile 26: per-half closers/epilogue + the
                        # DRAM-free b=7 softmax tail
                        s7_ps = pspool.tile([N, 1], F32, tag="sm", bufs=2)
                        rcpg_ps = pspool.tile([128, 4], F32, tag="sm", bufs=2)
                        oh = oneh_s[:, rt * RT:(rt + 1) * RT]
                        relu_w2 = work.tile([128, HID], F32, tag="relu_w2")
                        emit_vmms(t, ps, half=0)
                        mm(ps[:, 0:512], oh, hq_s[:, 0:512],
                           start=False, stop=True)
                        nc.vector.scalar_tensor_tensor(
                            out=relu_w2[:, 0:512], in0=ps[:, 0:512],
                            scalar=0.0, in1=w2b_s[:, 0:512],
                            op0=MAX, op1=MULT, accum_out=lsA26)
                        emit_vmms(t, ps, half=1)
                        mm(ps[:, 512:1024], oh, hq_s[:, 512:1024],
                           start=False, stop=True)
                        # per-n sums for tiles 23..25 (deps long ready)
                        for j in range(3):
                            mm(s7_ps, selT_s[:, j, :], e_ls[:, j:j + 1],
                               start=(j == 0), stop=False)
                        nc.vector.scalar_tensor_tensor(
                            out=relu_w2[:, 512:1024], in0=ps[:, 512:1024],
                            scalar=0.0, in1=w2b_s[:, 512:1024],
                            op0=MAX, op1=MULT, accum_out=lsB26)
                        # exp(lsA + lsB) fuses the halves via the ACT bias
                        nc.scalar.activation(e_ls[:, 3:4], lsB26, EXPF,
                                             bias=lsA26)
                        mm(s7_ps, selT_s[:, 3, :], e_ls[:, 3:4],
                           start=False, stop=True)
                        with nc.allow_low_precision(
                                reason="bf16 reciprocal of softmax sums: "
                                       "0.4% is within rel-err budget"):
                            nc.vector.reciprocal(rcp_ext[0:N, :], s7_ps)
                        for j in range(4):
                            mm(rcpg_ps[:, j:j + 1], selTp_s[:, j, :], rcp_ext,
                               start=True, stop=True)
                        nc.vector.scalar_tensor_tensor(
                            out=w_ls32[:, 0:4], in0=e_ls, scalar=0.0,
                            in1=rcpg_ps, op0=BYPASS, op1=MULT)
                        # 32-block transpose so every output DMA is a few
                        # FAT descriptors (a [128,3]-strided write costs
                        # ~22ns/descriptor in serialized acks = 8.5us for
                        # 384 descriptors -- measured)
                        nc.vector.transpose(w_t32, w_ls32)
                        # w_t32[32i + c, pl] = w_ls32[32i + pl, c] = w for
                        # output row 2944 + c*128 + 32i + pl; as 32-element
                        # runs at stride 128 these are step-slices of a
                        # [108, 32] view of out.  Rows < 3008 belong to the
                        # b0..6 path, so i=0/1 skip c=0.
                        ov32 = out.rearrange("(m q) -> m q", m=108, q=32)
                        nc.sync.dma_start(out=ov32[96:105:4],
                                          in_=w_t32[1:4, :])
                        nc.scalar.dma_start(out=ov32[97:106:4],
                                            in_=w_t32[33:36, :])
                        nc.gpsimd.dma_start(out=ov32[94:107:4],
                                            in_=w_t32[64:68, :])
                        nc.scalar.dma_start(out=ov32[95:108:4],
                                            in_=w_t32[96:100, :])
                vt_cur = vt_next

    nc.compile()
    return nc


def _get_nc():
    global _NC_CACHE
    if _NC_CACHE is None:
        _NC_CACHE = _build_nc()
    return _NC_CACHE


def _prepare_in_maps(inputs):
    v = np.asarray(inputs["v"], dtype=np.float32)
    q = np.asarray(inputs["q"], dtype=np.float32)
    W1 = np.asarray(inputs["W1"], dtype=np.float32)
    g1 = np.float64(np.asarray(inputs["g1"]))
    b1 = np.asarray(inputs["b1"], dtype=np.float32)
    W2 = np.asarray(inputs["W2"], dtype=np.float32)
    g2 = np.float64(np.asarray(inputs["g2"]))
    # b2 is a scalar added to every logit -> cancels in softmax over k.

    W1e = ((g1 / np.linalg.norm(W1.astype(np.float64))) * W1).astype(np.float32)
    W2e = ((g2 / np.linalg.norm(W2.astype(np.float64))) * W2).astype(np.float32)

    BF = ml_dtypes.bfloat16
    F8 = ml_dtypes.float8_e4m3   # TRN FP8_EXP4 (max ±240, has inf)
    # partition-major repacks: [..., 128 p, chunk, inner]
    w1vt = np.ascontiguousarray(                       # [128, 16, 1024]
        (W1e[:, :VD] * S1).T.reshape(CC, 128, HID).transpose(1, 0, 2)).astype(F8)
    w1qt = (W1e[:, VD:] * S1).T.reshape(QC, 128, HID).transpose(1, 0, 2)  # [128, 8, 1024]
    r = np.arange(R)
    oneh = (np.arange(BK)[:, None] == (r // N)[None, :]).astype(np.float32)
    selb = (np.arange(BL)[None, :] == (np.arange(BK) // K)[:, None]).astype(np.float32)

    w2bf = np.broadcast_to((W2e.reshape(1, HID) * (1.0 / S1)), (128, HID))
    b1bf = np.broadcast_to((b1.reshape(1, HID) * S1), (BK, HID))
    sel = np.zeros((BK, BL + BK), dtype=np.float32)
    sel[:, 0:BL] = selb
    sel[0:BL, BL:BL + BK] = selb.T

    selT = np.zeros((128, 4, N), dtype=np.float32)
    selTp = np.zeros((100, 4, 128), dtype=np.float32)
    for j in range(4):
        rr = (23 + j) * RT + np.arange(128)
        valid = rr >= B7R0
        pp = np.arange(128)[valid]
        nn = (rr % N)[valid]
        selT[pp, j, nn] = 1.0
        selTp[nn, j, pp] = 1.0
    # tile 23 rows 3008..3023 (p 64..79) belong to b6: gather 1/S_b6 from
    # rcp_ext's second half
    p6 = np.arange(64, 80)
    selTp[64 + ((2944 + p6) % N), 0, p6] = 1.0

    shared = dict(w1vt=w1vt, oneh=oneh, selTp=selTp.astype(BF),
                  w2b=np.ascontiguousarray(w2bf).astype(BF),
                  b1b=np.ascontiguousarray(b1bf).astype(BF), sel=sel,
                  selT=selT)
    in_maps = []
    for c in range(NCORES):
        vl = v[c * BL:(c + 1) * BL].reshape(R, VD)
        # vt[ch, p, cc, r_in_chunk] = v[ch*RC + r, cc*128 + p]
        vt4 = np.ascontiguousarray(
            vl.T.reshape(CC, 128, NCH, RC).transpose(2, 1, 0, 3)).astype(F8)
        ql = q[c * BL:(c + 1) * BL].reshape(BK, QD)
        qt3 = ql.T.reshape(QC, 128, BK).transpose(1, 0, 2)   # [128, 8, 96]
        qtwq = np.concatenate([qt3, w1qt], axis=2)           # [128, 8, 1120]
        in_maps.append(dict(vth=np.ascontiguousarray(vt4[:2]),
                            vtr=np.ascontiguousarray(vt4[2:]),
                            qtwq=np.ascontiguousarray(qtwq).astype(F8),
                            **shared))
    return in_maps


def kernel(**inputs) -> np.ndarray:
    in_maps = _prepare_in_maps(inputs)
    nc = _get_nc()
    res = run_bass_kernel_spmd(nc, in_maps, list(range(NCORES)))
    outs = [res.results[c]["out"].reshape(BL, K, N, 1) for c in range(NCORES)]
    return np.concatenate(outs, axis=0)
